# revision 1
# baseline (speedup 1.0000x reference)
"""ChildSum TreeLSTM (relational) — Trainium2 Bass kernel, 8 NeuronCores.

Strategy (data-parallel over batch, per sharding hint):
  - 16 trees are split over 8 cores, 2 whole trees per core.
  - Inside each core, nodes are relabeled level-by-level (sorted by tree
    height) so each bottom-up level occupies a contiguous row range of a
    padded node space.  All per-level gathers/scatters become small
    matmuls against host-built 0/1 incidence matrices (exact in fp).
  - Levels are processed one target 128-partition tile at a time with
    full-width engine ops; commits into the h/c state are masked with
    copy_predicated (engine APs may only start at partition 0/32/64/96,
    so arbitrary row slices are not addressable — full-width ops cost
    the same since engine time only scales with the free dimension).
  - Embedding rows are gathered on-device with indirect DMA from the
    replicated emb/rel tables; LSTM weights are replicated to every core.
  - Per-core output is the [12, trees_per_core] logits; the host
    assembles the [16, 12] result.

The SPMD program is identical on all cores; per-core behavior differs
only through input data (index vectors + incidence matrices).  Level
sizes are padded to the max across cores.
"""

import os
import numpy as np

P = 128
H = 256
HT = H // P          # h-state partition tiles
G3 = 3 * H           # packed i|o|u width (768)
N_CORES = 8


# ----------------------------------------------------------------------------
# Host-side plan builder
# ----------------------------------------------------------------------------

def _ceil_to(x, m):
    return (x + m - 1) // m * m


def _split_chunks(row0, cnt):
    """Split a row range into pieces that don't straddle 128-partition tiles."""
    out = []
    r, remaining = row0, cnt
    while remaining > 0:
        take = min(P - (r % P), remaining)
        out.append((r, take))
        r += take
        remaining -= take
    return out


def build_plan(xs, rels, child_idx, parent_idx, node_height, n_levels,
               n_cores=N_CORES):
    xs = np.asarray(xs)
    rels = np.asarray(rels)
    B, S = xs.shape
    tpc = B // n_cores
    heights = np.asarray(node_height).reshape(B, S)
    ci = np.asarray(child_idx)
    pi = np.asarray(parent_idx)
    NL = min(int(heights.max()) + 1, int(n_levels))

    edges_by_parent = {}
    for c, p in zip(ci.tolist(), pi.tolist()):
        edges_by_parent.setdefault(p, []).append(c)

    core_nodes, core_edges = [], []
    for core in range(n_cores):
        nl = [[] for _ in range(NL)]
        el = [[] for _ in range(NL)]
        for t in range(tpc):
            b = core * tpc + t
            for s in range(S):
                h = int(heights[b, s])
                if h < NL:
                    nl[h].append((t, s))
        for lv in range(1, NL):
            for (t, s) in nl[lv]:
                pg = (core * tpc + t) * S + s
                for cg in edges_by_parent.get(pg, []):
                    el[lv].append((cg, pg))
        core_nodes.append(nl)
        core_edges.append(el)

    n_hat = [max(len(core_nodes[c][lv]) for c in range(n_cores)) for lv in range(NL)]
    e_hat = [max(len(core_edges[c][lv]) for c in range(n_cores)) for lv in range(NL)]
    n_off = [0]
    for v in n_hat:
        n_off.append(n_off[-1] + v)
    e_off = [0]
    for v in e_hat:
        e_off.append(e_off[-1] + v)
    Npad = max(P, _ceil_to(n_off[-1], P))
    Epad = max(P, _ceil_to(e_off[-1], P))
    NKT, NET = Npad // P, Epad // P

    edge_chunks = [_split_chunks(e_off[lv], e_hat[lv]) for lv in range(NL)]
    # target node ptiles per level
    kts = [sorted({r // P for (r, c) in _split_chunks(n_off[lv], n_hat[lv])})
           for lv in range(NL)]

    # commit masks, uniform across cores: one [P,1] mask per (level, ptile)
    mask_idx = {}
    mask_rows = []
    for lv in range(NL):
        for kN in kts[lv]:
            m = np.zeros((P, 1), np.uint8)
            lo = max(n_off[lv], kN * P)
            hi = min(n_off[lv] + n_hat[lv], (kN + 1) * P)
            m[lo - kN * P:hi - kN * P, 0] = 1
            mask_idx[(lv, kN)] = len(mask_rows)
            mask_rows.append(m)
    masks = np.stack(mask_rows, axis=0) if mask_rows else np.zeros((1, P, 1), np.uint8)
    NM = masks.shape[0]

    # packed per-(level, edge-chunk, target-ptile) scatter blocks:
    # rows = chunk-local edge, cols = full 128 node slots of the target ptile
    afc_col = {}
    ac = 0
    for lv in range(1, NL):
        for ec_i in range(len(edge_chunks[lv])):
            for kN in kts[lv]:
                afc_col[(lv, ec_i, kN)] = ac
                ac += P
    AC = max(ac, 1)

    per_core = []
    for core in range(n_cores):
        slot_of = {}
        xs_idx = np.zeros((Npad, 1), np.int32)
        rel_idx = np.zeros((Npad, 1), np.int32)
        for lv in range(NL):
            for j, (t, s) in enumerate(core_nodes[core][lv]):
                slot = n_off[lv] + j
                g = (core * tpc + t) * S + s
                slot_of[g] = slot
                b = core * tpc + t
                xs_idx[slot, 0] = xs[b, s]
                rel_idx[slot, 0] = rels[b, s]
        G = np.zeros((NKT, P, Epad), np.float32)
        Gp = np.zeros((NKT, P, Epad), np.float32)
        Adj = np.zeros((NKT, P, Npad), np.float32)
        AfcL = np.zeros((P, AC), np.float32)
        Pperm = np.zeros((NKT, P, tpc * S), np.float32)
        for lv in range(1, NL):
            for j, (cg, pg) in enumerate(core_edges[core][lv]):
                e = e_off[lv] + j
                cs, ps = slot_of[cg], slot_of[pg]
                G[cs // P, cs % P, e] = 1.0
                Gp[ps // P, ps % P, e] = 1.0
                Adj[cs // P, cs % P, ps] = 1.0
                for ci_, (erow, ecnt) in enumerate(edge_chunks[lv]):
                    if erow <= e < erow + ecnt:
                        a0 = afc_col[(lv, ci_, ps // P)]
                        AfcL[e - erow, a0 + ps % P] = 1.0
                        break
        for g, slot in slot_of.items():
            t = g // S - core * tpc
            s = g % S
            Pperm[slot // P, slot % P, t * S + s] = 1.0
        per_core.append(dict(xs_idx=xs_idx, rel_idx=rel_idx, G=G, Gp=Gp,
                             Adj=Adj, AfcL=AfcL, Pperm=Pperm))

    # SPMD-uniform nonzero-block flags (OR across cores), level-exact columns
    gnz = np.zeros((NL, NKT), bool)
    bnz = np.zeros((NL, NKT), bool)
    for lv in range(1, NL):
        esl = slice(e_off[lv], e_off[lv] + e_hat[lv])
        nsl = slice(n_off[lv], n_off[lv] + n_hat[lv])
        for k in range(NKT):
            gnz[lv, k] = any(per_core[c]["G"][k, :, esl].any()
                             for c in range(n_cores))
            bnz[lv, k] = any(per_core[c]["Adj"][k, :, nsl].any()
                             for c in range(n_cores))
    gpnz = np.zeros((NET, NKT), bool)
    for ke in range(NET):
        esl = slice(ke * P, (ke + 1) * P)
        for k in range(NKT):
            gpnz[ke, k] = any(per_core[c]["Gp"][k, :, esl].any()
                              for c in range(n_cores))

    # combined gather blocks: per level (>=1), cols = [G-cols (even-padded) |
    # Adj-cols of each target ptile], padded to >=256 for full-rate fp32r
    ga_off, ga_w, ga_ec2 = {}, {}, {}
    ga_total = 0
    for lv in range(1, NL):
        ec2 = e_hat[lv] + (e_hat[lv] & 1)
        w = max(256, _ceil_to(ec2 + P * len(kts[lv]), 2))
        assert w <= 512, f"level {lv} gather block too wide ({w})"
        ga_ec2[lv] = ec2
        ga_off[lv] = ga_total
        ga_w[lv] = w
        ga_total += w
    for cd in per_core:
        GA = np.zeros((NKT, P, max(ga_total, 2)), np.float32)
        for lv in range(1, NL):
            o0, w0, ec2 = ga_off[lv], ga_w[lv], ga_ec2[lv]
            e0 = e_off[lv]
            for k in range(NKT):
                GA[k, :, o0:o0 + min(ec2, Epad - e0)] = \
                    cd["G"][k][:, e0:e0 + min(ec2, Epad - e0)]
                for i, kN in enumerate(kts[lv]):
                    blk = cd["Adj"][k][:, kN * P:(kN + 1) * P].copy()
                    lo = max(n_off[lv], kN * P) - kN * P
                    hi = min(n_off[lv] + n_hat[lv], (kN + 1) * P) - kN * P
                    blk[:, :lo] = 0.0
                    blk[:, hi:] = 0.0
                    GA[k, :, o0 + ec2 + i * P:o0 + ec2 + (i + 1) * P] = blk
        cd["GA"] = GA
    GAtot = max(ga_total, 2)
    kgb = np.zeros((NL, NKT), bool)
    for lv in range(1, NL):
        for k in range(NKT):
            kgb[lv, k] = any(
                per_core[c]["GA"][k][:, ga_off[lv]:ga_off[lv] + ga_w[lv]].any()
                for c in range(n_cores))

    sizes = dict(NL=NL, Npad=Npad, Epad=Epad, NKT=NKT, NET=NET, tpc=tpc, S=S,
                 AC=AC, NM=NM, n_hat=n_hat, e_hat=e_hat, n_off=n_off,
                 e_off=e_off, edge_chunks=edge_chunks, kts=kts,
                 mask_idx=mask_idx, masks=masks, afc_col=afc_col,
                 gnz=gnz, bnz=bnz, gpnz=gpnz, kgb=kgb,
                 ga_off=ga_off, ga_w=ga_w, ga_ec2=ga_ec2, GAtot=GAtot)

    # ---- packed constant column layout (f32 block + int block) ----
    TS = tpc * S
    cols = {}
    cptr = 0
    def _alloc(name, w):
        nonlocal cptr
        cols[name] = (cptr, w)
        cptr += w
    _alloc("bias", G3 + H + 16)       # row0: [bi768 | bf256 | bout]
    _alloc("ones", P)
    _alloc("ident", P)
    _alloc("zero", H)
    _alloc("relw64", P)
    _alloc("relh", Npad)
    for d in range(2):
        _alloc(f"wioux{d}", G3)
        _alloc(f"wfx{d}", H)
    for k2 in range(2):
        _alloc(f"wiouh{k2}", G3)
        _alloc(f"wfh{k2}", H)
        _alloc(f"wout{k2}", 16)
    for k in range(NKT):
        _alloc(f"GA{k}", GAtot)
    for k in range(NKT):
        _alloc(f"G{k}", Epad)
    for k in range(NKT):
        _alloc(f"Gp{k}", Epad)
    _alloc("Afc", AC + (AC & 1))
    for k in range(NKT):
        _alloc(f"Pp{k}", TS)
    sizes["cols"] = cols
    sizes["C"] = cptr
    icols = {}
    iptr = 0
    def _ialloc(name, w):
        nonlocal iptr
        icols[name] = (iptr, w)
        iptr += w
    _ialloc("xsidx", NKT)
    _ialloc("relidx", NKT)
    _ialloc("masks", NM)
    sizes["icols"] = icols
    sizes["CI"] = iptr
    return sizes, per_core


def pack_weights(inp):
    f32 = np.float32
    a = lambda k: np.asarray(inp[k], f32)
    WiouX = np.ascontiguousarray(
        np.concatenate([a("W_ix"), a("W_ox"), a("W_ux")], axis=1))   # [DIN,768]
    WiouH = np.ascontiguousarray(
        np.concatenate([a("W_ih"), a("W_oh"), a("W_uh")], axis=1))   # [H,768]
    bi512 = np.zeros((1, 512), f32)
    bi512[0, :H] = a("b_ix") + a("b_ih")
    bf = np.ascontiguousarray((a("b_fx") + a("b_fh")).reshape(1, H))
    return WiouX, WiouH, bi512, bf


# ----------------------------------------------------------------------------
# Numpy emulation of the device program (validation / fallback)
# ----------------------------------------------------------------------------

def emulate_core(sizes, cd, emb_W, rel_W, WiouX, WiouH, Wfx, Wfh,
                 bi512, bf, Wout, bout):
    f32 = np.float32
    NL, NKT = sizes["NL"], sizes["NKT"]
    Npad = sizes["Npad"]
    x = np.concatenate([emb_W[cd["xs_idx"][:, 0]], rel_W[cd["rel_idx"][:, 0]]],
                       axis=1).astype(f32)
    iou_x = (x @ WiouX).astype(f32)
    fx = (x @ Wfx).astype(f32)
    GpF = np.concatenate([cd["Gp"][k] for k in range(NKT)], axis=0)
    fxe = (GpF.T @ fx).astype(f32)
    GF = np.concatenate([cd["G"][k] for k in range(NKT)], axis=0)
    AdjF = np.concatenate([cd["Adj"][k] for k in range(NKT)], axis=0)
    h = np.zeros((Npad, H), f32)
    c = np.zeros((Npad, H), f32)
    bi_full = np.concatenate([bi512[0], np.zeros(G3 - 512, f32)])

    def sigmoid(v):
        return (1.0 / (1.0 + np.exp(-v.astype(f32)))).astype(f32)

    for lv in range(NL):
        fc_full = {kN: np.zeros((P, H), f32) for kN in sizes["kts"][lv]}
        fce_buf = np.zeros((P, H), f32)
        if lv > 0:
            for ec_i, (erow, ecnt) in enumerate(sizes["edge_chunks"][lv]):
                Gl = GF[:, erow:erow + ecnt]
                hgT = (h.T @ Gl).astype(f32)
                cg = (Gl.T @ c).astype(f32)
                fpre = (hgT.T @ Wfh).astype(f32) + fxe[erow:erow + ecnt] + bf[0]
                fce_buf = fce_buf.copy()
                fce_buf[:ecnt] = (sigmoid(fpre) * cg).astype(f32)
                for kN in sizes["kts"][lv]:
                    a0 = sizes["afc_col"][(lv, ec_i, kN)]
                    Af = cd["AfcL"][:, a0:a0 + P]
                    fc_full[kN] += (Af.T @ fce_buf).astype(f32)
        for kN in sizes["kts"][lv]:
            if lv > 0:
                hsT = (h.T @ AdjF[:, kN * P:(kN + 1) * P]).astype(f32)
                iou = (hsT.T @ WiouH).astype(f32) + iou_x[kN * P:(kN + 1) * P] \
                    + bi_full
            else:
                iou = iou_x[kN * P:(kN + 1) * P] + bi_full
            i = sigmoid(iou[:, 0:H])
            og = sigmoid(iou[:, H:2 * H])
            u = np.tanh(iou[:, 2 * H:]).astype(f32)
            cn = (i * u + fc_full[kN]).astype(f32)
            hn = (og * np.tanh(cn)).astype(f32)
            m = sizes["masks"][sizes["mask_idx"][(lv, kN)]][:, 0] > 0
            c[kN * P:(kN + 1) * P][m] = cn[m]
            h[kN * P:(kN + 1) * P][m] = hn[m]

    PF = np.concatenate([cd["Pperm"][k] for k in range(NKT)], axis=0)
    hT_ord = (h.T @ PF).astype(f32)
    S = sizes["S"]
    pooled = np.stack([hT_ord[:, t * S:(t + 1) * S].max(axis=1)
                       for t in range(sizes["tpc"])], axis=1)
    return (Wout.T @ pooled).astype(f32) + bout[:, None]      # [12, tpc]


def kernel_numpy(**inputs):
    sizes, per_core = build_plan(inputs["xs"], inputs["rels"],
                                 inputs["child_idx"], inputs["parent_idx"],
                                 inputs["node_height"], int(inputs["n_levels"]))
    WiouX, WiouH, bi512, bf = pack_weights(inputs)
    emb_W = np.asarray(inputs["emb_W"], np.float32)
    rel_W = np.asarray(inputs["rel_W"], np.float32)
    outs = []
    for cd in per_core:
        lT = emulate_core(sizes, cd, emb_W, rel_W, WiouX, WiouH,
                          np.asarray(inputs["W_fx"], np.float32),
                          np.asarray(inputs["W_fh"], np.float32),
                          bi512, bf,
                          np.asarray(inputs["W_out"], np.float32),
                          np.asarray(inputs["b_out"], np.float32))
        outs.append(lT.T)
    return np.concatenate(outs, axis=0).astype(np.float32)


# ----------------------------------------------------------------------------
# Device program
# ----------------------------------------------------------------------------

def build_bass(sizes, V, DE, RV, DR, L, wdt="f32r"):
    from concourse import bacc, bass, mybir, tile

    f32 = mybir.dt.float32
    f32r = mybir.dt.float32r
    i32 = mybir.dt.int32
    WD = f32r if wdt == "f32r" else mybir.dt.bfloat16
    SIG = mybir.ActivationFunctionType.Sigmoid
    TANH = mybir.ActivationFunctionType.Tanh
    AXX = mybir.AxisListType.X

    NL, Npad, Epad = sizes["NL"], sizes["Npad"], sizes["Epad"]
    NKT, NET, tpc, S = sizes["NKT"], sizes["NET"], sizes["tpc"], sizes["S"]
    NM, C, CI = sizes["NM"], sizes["C"], sizes["CI"]
    cols, icols = sizes["cols"], sizes["icols"]
    DIN = DE + DR
    DT = DIN // P
    TS = tpc * S

    nc = bacc.Bacc("TRN2", target_bir_lowering=False, debug=False)

    d_emb = nc.dram_tensor("emb", [V, DE], f32, kind="ExternalInput")
    d_rel = nc.dram_tensor("relw", [RV, DR], f32, kind="ExternalInput")
    d_bigc = nc.dram_tensor("bigc", [P, C], f32 if wdt == "f32r" else WD,
                            kind="ExternalInput")
    d_idf = nc.dram_tensor("identf", [P, P], f32, kind="ExternalInput")
    d_bigi = nc.dram_tensor("bigi", [P, max(CI, 1)], i32, kind="ExternalInput")
    d_out = nc.dram_tensor("out", [L, tpc], f32, kind="ExternalOutput")

    with tile.TileContext(nc) as tc:
        with (
            tc.tile_pool(name="const", bufs=1) as cp,
            tc.tile_pool(name="psg", bufs=2, space="PSUM") as ps_g,
            tc.tile_pool(name="psm", bufs=1, space="PSUM") as ps_m,
            tc.tile_pool(name="psfc", bufs=2, space="PSUM") as ps_fc,
            tc.tile_pool(name="psiou", bufs=1, space="PSUM") as ps_iou,
        ):
            t = lambda shape, dt_, tag: cp.tile(shape, dt_, tag=tag, name=tag)
            bigc = t([P, C], WD, "bigc")
            bigi = t([P, max(CI, 1)], i32, "bigi")

            def cc(name):
                off, w = cols[name]
                return bigc[:, off:off + w]

            def ci(name, j):
                off, _ = icols[name]
                return bigi[:, off + j:off + j + 1]

            wioux = [cc(f"wioux{d}") for d in range(DT)]
            wiouh = [cc(f"wiouh{k}") for k in range(HT)]
            wfx = [cc(f"wfx{d}") for d in range(DT)]
            wfh = [cc(f"wfh{k}") for k in range(HT)]
            wout = [cc(f"wout{k}")[:, :L] for k in range(HT)]
            boff = cols["bias"][0]
            bi_row = bigc[0:1, boff:boff + 512]
            bf_row = bigc[0:1, boff + G3:boff + G3 + H]
            bout_row = bigc[0:1, boff + G3 + H:boff + G3 + H + L]
            ones_row = bigc[0:1, cols["ones"][0]:cols["ones"][0] + P]
            identr = cc("ident")
            identf = t([P, P], f32, "identf")
            zeror = cc("zero")
            Gsb = [cc(f"G{k}") for k in range(NKT)]
            Gpsb = [cc(f"Gp{k}") for k in range(NKT)]
            Afcsb = cc("Afc")
            Ppsb = [cc(f"Pp{k}") for k in range(NKT)]

            xsb = [t([P, DE], f32, f"x{k}") for k in range(NKT)]
            xT = [t([P, Npad], WD, f"xT{d}") for d in range(DT)]
            ioux = [t([P, G3], WD, f"ioux{k}") for k in range(NKT)]
            fxsb = [t([P, H], WD, f"fx{k}") for k in range(NKT)]
            fxesb = [t([P, H], WD, f"fxe{e}") for e in range(NET)]
            hsb = [[t([P, P], f32 if wdt == "f32r" else WD, f"h{k}_{kh}")
                    for kh in range(HT)] for k in range(NKT)]
            hrb = ([[t([P, P], f32r, f"hr{k}_{kh}") for kh in range(HT)]
                    for k in range(NKT)]
                   if wdt == "f32r" else hsb)
            csb = [t([P, H], f32 if wdt == "f32r" else WD, f"c{k}") for k in range(NKT)]
            crb = ([t([P, H], f32r, f"cr{k}") for k in range(NKT)]
                   if wdt == "f32r" else csb)
            hgst = [t([P, 512], WD, f"hgst{k}") for k in range(HT)]
            fgate = t([P, H], WD, "fgate")
            fce = t([P, H], WD, "fce")
            iosb = t([P, 512], f32, "iosb")
            usb = t([P, H], f32, "usb")
            cnew = t([P, H], f32, "cnew")
            thsb = t([P, H], f32, "thsb")
            hnew = t([P, H], f32, "hnew")
            pooled = [t([P, tpc], WD, f"pool{k}") for k in range(HT)]
            outsb = t([L, tpc], f32, "outsb")

            # ---- preamble loads: int block first, then f32 block in need
            # order (each dma lands on its own queue; 128 descriptors each)
            bct = (lambda ap: ap.bitcast(f32r)) if wdt == "f32r" else (lambda ap: ap)
            nc.sync.dma_start(bigi[:], d_bigi[:])
            nc.sync.dma_start(identf[:], d_idf[:])
            misc_end = cols["zero"][0] + cols["zero"][1]
            nc.sync.dma_start(bigc[:, 0:misc_end],
                              bct(d_bigc[:, 0:misc_end]))
            wx_end = cols["wfx1"][0] + cols["wfx1"][1]
            nc.sync.dma_start(bigc[:, misc_end:wx_end],
                              bct(d_bigc[:, misc_end:wx_end]))
            wh_end = cols["wout1"][0] + cols["wout1"][1]
            nc.sync.dma_start(bigc[:, wx_end:wh_end],
                              bct(d_bigc[:, wx_end:wh_end]))
            ga_end = cols[f"GA{NKT-1}"][0] + cols[f"GA{NKT-1}"][1]
            nc.sync.dma_start(bigc[:, wh_end:ga_end],
                              bct(d_bigc[:, wh_end:ga_end]))
            g_end = cols[f"Gp{NKT-1}"][0] + cols[f"Gp{NKT-1}"][1]
            nc.sync.dma_start(bigc[:, ga_end:g_end],
                              bct(d_bigc[:, ga_end:g_end]))
            nc.sync.dma_start(bigc[:, g_end:C], bct(d_bigc[:, g_end:C]))

            for k in range(NKT):
                for kh in range(HT):
                    nc.gpsimd.memset(hsb[k][kh][:], 0.0)
                nc.gpsimd.memset(csb[k][:], 0.0)
                if wdt == "f32r":
                    for kh in range(HT):
                        nc.vector.tensor_copy(out=hrb[k][kh][:],
                                              in_=zeror[:, 0:P])
                    nc.vector.tensor_copy(out=crb[k][:], in_=zeror)
            nc.vector.tensor_copy(out=fce[:], in_=zeror)

            # ---- embedding gathers (emb via indirect DMA; rel rows via a
            # one-hot matmul against the SBUF-resident 64-row rel table,
            # which lands directly transposed into xT[1])
            for k in range(NKT):
                nc.gpsimd.indirect_dma_start(
                    out=xsb[k][:, 0:DE], out_offset=None, in_=d_emb[:],
                    in_offset=bass.IndirectOffsetOnAxis(ap=ci("xsidx", k), axis=0))
            relw_off = cols["relw64"][0]
            relh_off = cols["relh"][0]
            prl = ps_m.tile([P, Npad], f32, tag="cg", name="cg")
            nc.tensor.matmul(prl[:, :],
                             lhsT=bigc[0:RV, relw_off:relw_off + P],
                             rhs=bigc[0:RV, relh_off:relh_off + Npad],
                             start=True, stop=True)
            nc.vector.tensor_copy(out=xT[1][P - DR:P, :],
                                  in_=prl[P - DR:P, :])

            # ---- x transpose (emb part only: DE = 192 = 128 + 64 cols)
            for k in range(NKT):
                pt = ps_m.tile([P, P], f32, tag="cg", name="cg")
                nc.tensor.transpose(pt[:], xsb[k][:, 0:P], identf[:])
                nc.vector.tensor_copy(out=xT[0][:, k * P:(k + 1) * P],
                                      in_=pt[:])
                pt2 = ps_m.tile([P, P], f32, tag="cg", name="cg")
                nc.tensor.transpose(pt2[:DE - P, :], xsb[k][:, P:DE],
                                    identf[:])
                nc.vector.tensor_copy(out=xT[1][0:DE - P, k * P:(k + 1) * P],
                                      in_=pt2[:DE - P, :])

            # ---- input projections (biases folded in via ones-row matmul)
            for k in range(NKT):
                pi = ps_iou.tile([P, G3], f32, tag="iou", name="iou")
                for c0, cn_ in ((0, 512), (512, G3 - 512)):
                    has_bias = (c0 == 0)
                    for d in range(DT):
                        nc.tensor.matmul(
                            pi[:, c0:c0 + cn_],
                            lhsT=xT[d][:, k * P:(k + 1) * P],
                            rhs=wioux[d][:, c0:c0 + cn_],
                            start=(d == 0),
                            stop=(not has_bias and d == DT - 1))
                    if has_bias:
                        nc.tensor.matmul(pi[:, 0:512], lhsT=ones_row,
                                         rhs=bi_row, start=False, stop=True)
                nc.vector.tensor_copy(out=ioux[k][:], in_=pi[:])
                pf = ps_m.tile([P, H], f32, tag="fp", name="fp")
                for d in range(DT):
                    nc.tensor.matmul(pf[:], lhsT=xT[d][:, k * P:(k + 1) * P],
                                     rhs=wfx[d][:], start=(d == 0), stop=False)
                nc.tensor.matmul(pf[:], lhsT=ones_row, rhs=bf_row,
                                 start=False, stop=True)
                nc.vector.tensor_copy(out=fxsb[k][:], in_=pf[:])

            # ---- fxe: fx gathered per edge slot
            for ke in range(NET):
                ks = [k for k in range(NKT) if sizes["gpnz"][ke, k]]
                if not ks:
                    nc.vector.tensor_copy(out=fxesb[ke][:], in_=zeror)
                    continue
                pf = ps_m.tile([P, H], f32, tag="fp", name="fp")
                for i, k in enumerate(ks):
                    nc.tensor.matmul(pf[:],
                                     lhsT=Gpsb[k][:, ke * P:(ke + 1) * P],
                                     rhs=fxsb[k][:],
                                     start=(i == 0), stop=(i == len(ks) - 1))
                nc.vector.tensor_copy(out=fxesb[ke][:], in_=pf[:])

            # ---- levels
            for lv in range(NL):
                kts = sizes["kts"][lv]
                fc_ps = {}
                if lv > 0:
                    prev = set(sizes["kts"][lv - 1])
                    okey = lambda k: (k in prev, k)
                    kg = sorted((k for k in range(NKT) if sizes["gnz"][lv, k]),
                                key=okey)
                    kgb = sorted((k for k in range(NKT) if sizes["kgb"][lv, k]),
                                 key=okey)
                    echunks = sizes["edge_chunks"][lv]
                    ga0l = sizes["ga_off"][lv]
                    gawl = sizes["ga_w"][lv]
                    ec2 = sizes["ga_ec2"][lv]
                    # combined gather: h_children^T | h_sum^T per H-ptile
                    for kh in range(HT):
                        pg = ps_g.tile([P, 512], f32, tag="gst", name="gst")
                        for i, k in enumerate(kgb):
                            nc.tensor.matmul(
                                pg[:, :gawl],
                                lhsT=hrb[k][kh][:],
                                rhs=bigc[:, cols[f"GA{k}"][0] + ga0l:
                                         cols[f"GA{k}"][0] + ga0l + gawl],
                                start=(i == 0), stop=(i == len(kgb) - 1))
                        nc.vector.tensor_copy(out=hgst[kh][:, :gawl],
                                              in_=pg[:, :gawl])
                    for ec_i, (erow, ecnt) in enumerate(echunks):
                        ke, r0e = erow // P, erow % P
                        eloc = erow - sizes["e_off"][lv]
                        # c_children (edge-major)
                        pc = ps_m.tile([P, H], f32, tag="cg", name="cg")
                        for i, k in enumerate(kg):
                            nc.tensor.matmul(
                                pc[:ecnt, :],
                                lhsT=Gsb[k][:, erow:erow + ecnt],
                                rhs=crb[k][:],
                                start=(i == 0), stop=(i == len(kg) - 1))
                        # f preactivation = h_ch @ Wfh + fxe  (bias in fx)
                        pfp = ps_m.tile([P, H], f32, tag="fp", name="fp")
                        for kh in range(HT):
                            nc.tensor.matmul(pfp[:ecnt, :],
                                             lhsT=hgst[kh][:, eloc:eloc + ecnt],
                                             rhs=wfh[kh][:],
                                             start=(kh == 0), stop=False)
                        nc.tensor.matmul(pfp[:ecnt, :],
                                         lhsT=identr[:, r0e:r0e + ecnt],
                                         rhs=fxesb[ke][:],
                                         start=False, stop=True)
                        nc.scalar.activation(fgate[:ecnt, :], pfp[:ecnt, :], SIG)
                        nc.vector.tensor_mul(fce[:ecnt, :],
                                             fgate[:ecnt, :], pc[:ecnt, :])
                        first = (ec_i == 0)
                        last = (ec_i == len(echunks) - 1)
                        for kN in kts:
                            if first:
                                fc_ps[kN] = ps_fc.tile([P, H], f32, tag="fc",
                                                       name="fc")
                            a0 = sizes["afc_col"][(lv, ec_i, kN)]
                            nc.tensor.matmul(
                                fc_ps[kN][:],
                                lhsT=Afcsb[:, a0:a0 + P],
                                rhs=fce[:],
                                start=first, stop=last)

                # i/o/u per target ptile
                for kti, kN in enumerate(kts):
                    if lv > 0:
                        hoff = sizes["ga_ec2"][lv] + kti * P
                        pi = ps_iou.tile([P, G3], f32, tag="iou", name="iou")
                        for c0, cn_ in ((0, 512), (512, G3 - 512)):
                            for kh in range(HT):
                                nc.tensor.matmul(
                                    pi[:, c0:c0 + cn_],
                                    lhsT=hgst[kh][:, hoff:hoff + P],
                                    rhs=wiouh[kh][:, c0:c0 + cn_],
                                    start=(kh == 0), stop=False)
                            nc.tensor.matmul(
                                pi[:, c0:c0 + cn_],
                                lhsT=identr[:, :P],
                                rhs=ioux[kN][:, c0:c0 + cn_],
                                start=False, stop=True)
                        nc.scalar.activation(iosb[:, 0:H], pi[:, 0:H], SIG)
                        nc.scalar.activation(usb[:], pi[:, 512:G3], TANH)
                        nc.scalar.activation(iosb[:, H:512], pi[:, H:512], SIG)
                    else:
                        iax = (ioux[kN][:].bitcast(f32) if wdt == "f32r"
                               else ioux[kN][:])
                        nc.scalar.activation(iosb[:, 0:H], iax[:, 0:H], SIG)
                        nc.scalar.activation(usb[:], iax[:, 512:G3], TANH)
                        nc.scalar.activation(iosb[:, H:512], iax[:, H:512], SIG)
                    nc.vector.tensor_mul(cnew[:], iosb[:, 0:H], usb[:])
                    if lv > 0:
                        nc.vector.tensor_add(cnew[:], cnew[:], fc_ps[kN][:])
                    msk = ci("masks", sizes["mask_idx"][(lv, kN)])
                    # h-side tail split into 128-col halves so the next
                    # level's kh0 gather can start while kh1 still commits
                    for hh in range(HT):
                        hs = slice(hh * P, (hh + 1) * P)
                        nc.scalar.activation(thsb[:, hs], cnew[:, hs], TANH)
                        nc.vector.tensor_mul(hnew[:, hs],
                                             iosb[:, H + hh * P:H + (hh + 1) * P],
                                             thsb[:, hs])
                        nc.vector.copy_predicated(
                            out=hsb[kN][hh][:], mask=msk.to_broadcast([P, P]),
                            data=hnew[:, hs])
                        if wdt == "f32r":
                            nc.vector.tensor_copy(out=hrb[kN][hh][:],
                                                  in_=hsb[kN][hh][:])
                    nc.vector.copy_predicated(
                        out=csb[kN][:], mask=msk.to_broadcast([P, H]),
                        data=cnew[:])
                    if wdt == "f32r":
                        nc.scalar.copy(out=crb[kN][:], in_=csb[kN][:])
            # ---- readout
            plg = ps_m.tile([P, tpc], f32, tag="fp", name="fp")
            last_kts = set(sizes["kts"][NL - 1])
            ro_order = sorted(range(NKT), key=lambda k: (k in last_kts, k))
            for kh in range(HT):
                pr = ps_m.tile([P, TS], f32, tag="cg", name="cg")
                for i, k in enumerate(ro_order):
                    nc.tensor.matmul(pr[:],
                                     lhsT=hrb[k][kh][:],
                                     rhs=Ppsb[k][:],
                                     start=(i == 0), stop=(i == NKT - 1))
                for t_ in range(tpc):
                    nc.vector.reduce_max(pooled[kh][:, t_:t_ + 1],
                                         pr[:, t_ * S:(t_ + 1) * S], axis=AXX)
            for kh in range(HT):
                nc.tensor.matmul(plg[:L, :], lhsT=wout[kh],
                                 rhs=pooled[kh][:],
                                 start=(kh == 0), stop=False)
            nc.tensor.matmul(plg[:L, :], lhsT=bout_row,
                             rhs=ones_row[:, :tpc], start=False, stop=True)
            nc.vector.tensor_copy(out=outsb[:], in_=plg[:L, :])
            nc.sync.dma_start(d_out[:, :], outsb[:])

    nc.compile()
    return nc


def _make_in_maps(sizes, per_core, inputs, wdt="f32r"):
    f32 = np.float32
    WiouX, WiouH, bi512, bf = pack_weights(inputs)
    cols, C = sizes["cols"], sizes["C"]
    icols, CI = sizes["icols"], sizes["CI"]
    NKT, NM = sizes["NKT"], sizes["NM"]
    L = np.asarray(inputs["W_out"]).shape[1]

    base = np.zeros((P, C), f32)

    def put(name, arr, row0=0):
        off, w = cols[name]
        arr = np.asarray(arr, f32)
        base[row0:row0 + arr.shape[0], off:off + arr.shape[1]] = arr

    for d in range(2):
        put(f"wioux{d}", WiouX[d * P:(d + 1) * P])
        put(f"wfx{d}", np.asarray(inputs["W_fx"], f32)[d * P:(d + 1) * P])
    for k2 in range(2):
        put(f"wiouh{k2}", WiouH[k2 * P:(k2 + 1) * P])
        put(f"wfh{k2}", np.asarray(inputs["W_fh"], f32)[k2 * P:(k2 + 1) * P])
        put(f"wout{k2}", np.asarray(inputs["W_out"], f32)[k2 * P:(k2 + 1) * P])
    brow = np.zeros((1, cols["bias"][1]), f32)
    brow[0, :512] = bi512[0]
    brow[0, G3:G3 + H] = bf[0]
    brow[0, G3 + H:G3 + H + L] = np.asarray(inputs["b_out"], f32)
    put("bias", brow)
    put("ones", np.ones((1, P), f32))
    put("ident", np.eye(P, dtype=f32))
    relW = np.asarray(inputs["rel_W"], f32)          # [R, DR]
    rw = np.zeros((relW.shape[0], P), f32)
    rw[:, P - relW.shape[1]:] = relW                 # rel dims land at rows 64:128
    put("relw64", rw)
    # "zero" block stays zero

    ibase = np.zeros((P, max(CI, 1)), np.int32)

    in_maps = []
    for cd in per_core:
        bc = base.copy()
        for k in range(NKT):
            off, w = cols[f"GA{k}"]
            bc[:, off:off + cd["GA"].shape[2]] = cd["GA"][k]
            off, w = cols[f"G{k}"]
            bc[:, off:off + w] = cd["G"][k]
            off, w = cols[f"Gp{k}"]
            bc[:, off:off + w] = cd["Gp"][k]
            off, w = cols[f"Pp{k}"]
            bc[:, off:off + w] = cd["Pperm"][k]
        off, w = cols["Afc"]
        bc[:, off:off + cd["AfcL"].shape[1]] = cd["AfcL"]
        off, w = cols["relh"]
        rh = np.zeros((P, w), f32)
        rh[cd["rel_idx"][:, 0], np.arange(w)] = 1.0
        bc[:, off:off + w] = rh
        bi_ = ibase.copy()
        xo = icols["xsidx"][0]
        ro = icols["relidx"][0]
        mo = icols["masks"][0]
        for k in range(NKT):
            bi_[:, xo + k] = cd["xs_idx"][k * P:(k + 1) * P, 0]
            bi_[:, ro + k] = cd["rel_idx"][k * P:(k + 1) * P, 0]
        for m in range(NM):
            bi_[:, mo + m] = sizes["masks"][m][:, 0].astype(np.int32)
        if wdt != "f32r":
            import ml_dtypes
            bc = bc.astype(ml_dtypes.bfloat16)
        in_maps.append(dict(
            emb=np.ascontiguousarray(np.asarray(inputs["emb_W"], f32)),
            relw=np.ascontiguousarray(np.asarray(inputs["rel_W"], f32)),
            bigc=np.ascontiguousarray(bc),
            bigi=np.ascontiguousarray(bi_),
            identf=np.eye(P, dtype=f32),
        ))
    return in_maps


def kernel(**inputs):
    sizes, per_core = build_plan(inputs["xs"], inputs["rels"],
                                 inputs["child_idx"], inputs["parent_idx"],
                                 inputs["node_height"], int(inputs["n_levels"]))
    V, DE = np.asarray(inputs["emb_W"]).shape
    RV, DR = np.asarray(inputs["rel_W"]).shape
    L = np.asarray(inputs["W_out"]).shape[1]
    wdt = os.environ.get("TREELSTM_WDT", "f32r")
    nc = build_bass(sizes, V, DE, RV, DR, L, wdt=wdt)
    in_maps = _make_in_maps(sizes, per_core, inputs, wdt=wdt)

    if os.environ.get("TREELSTM_SIM") == "1":
        from concourse.bass_interp import CoreSim
        outs = []
        for cid in range(N_CORES):
            sim = CoreSim(nc)
            for name, val in in_maps[cid].items():
                sim.tensor(name)[:] = val
            sim.simulate()
            outs.append(np.array(sim.tensor("out")).T)
        return np.concatenate(outs, axis=0).astype(np.float32)

    from concourse.bass_utils import run_bass_kernel_spmd
    res = run_bass_kernel_spmd(nc, in_maps, core_ids=list(range(N_CORES)),
                               trace=bool(int(os.environ.get("TREELSTM_TRACE", "0"))))
    if getattr(kernel, "_keep_results", False):
        kernel.last_results = res
    out = np.concatenate([r["out"].T for r in res.results], axis=0)
    return out.astype(np.float32)

